# revision 1
# baseline (speedup 1.0000x reference)
"""Trainium2 Bass kernel for nn_Attention_65317862638379.

Dense transformer block-attention with per-token geometric (rotation+translation)
transform. B=16, N=2048, DIM=1024, H=16, DH=64; attention over N/4=512 block
tokens of dim 256.

Sharding: data-parallel over batch, 2 batches per core, 8 cores, no collectives.
All matmuls run in bf16 on the PE (fp32 PSUM accumulation).

Layouts (per batch, per core):
  - Q/K: transposed projection -> qkT [j=(head,dh) rows, t] ; fwd rotation done
    with stream_shuffle (partition pair swap) + cosT/sinT coefficient tiles.
  - V: a-split natural projection -> Vb [J, (h,a,dh)] block layout; fwd rotation
    on the free axis + translation.
  - Attention simT[J, I] per head; softmax without max-subtraction (scores are
    O(5)); denominators via ones-matmul (replicated across partitions).
  - PV -> A [(a,dh), I]; normalize, inverse translate/rotate, write attn_outT
    [j, t]; final projection back to natural [t, e].
"""

import numpy as np
import ml_dtypes

import concourse.bass as bass
import concourse.mybir as mybir
import concourse.tile as tile
from concourse.bass_utils import run_bass_kernel_spmd

BF16 = ml_dtypes.bfloat16

B, N, DIM, H, DH = 16, 2048, 1024, 16, 64
D_FLAT, D_ROT, NPAIR = 32, 32, 16
BLK = 4
NB = N // BLK          # 512 block tokens
DB = DH * BLK          # 256 block dim
NCORES = 8
B2 = B // NCORES       # batches per core
SCALE = float((DH * BLK) ** -0.5)  # 1/16, TAU=1.0

FP32 = mybir.dt.float32
BFD = mybir.dt.bfloat16

MULT = mybir.AluOpType.mult
ADD = mybir.AluOpType.add
SUB = mybir.AluOpType.subtract

_CACHE = {}


def _split_multi_waits(nc):
    """walrus codegen only supports one sync-wait per instruction; hoist
    extra waits onto preceding same-engine NoOps."""
    cnt = 0
    for f in nc.m.functions:
        for blk in f.blocks:
            insts = blk.instructions
            out = []
            for inst in insts:
                si = inst.sync_info
                if si is not None and si.on_wait and len(si.on_wait) > 1:
                    waits = list(si.on_wait)
                    for w in waits[:-1]:
                        cnt += 1
                        nop = mybir.InstNoOp(name=f"WSPLIT-{cnt}", ins=[], outs=[])
                        nop.engine = inst.engine
                        nop.sync_info = mybir.SyncInfo(on_wait=[w], on_update=[])
                        out.append(nop)
                    inst.sync_info = mybir.SyncInfo(
                        on_wait=[waits[-1]], on_update=list(si.on_update))
                out.append(inst)
            blk.instructions = out
    return cnt


def _build_nc():
    """Build the Bass graph (SPMD; same NEFF on all 8 cores)."""
    nc = bass.Bass(target_bir_lowering=False)

    # ---------------- DRAM parameters (per-core shapes) ----------------
    xT_d = nc.dram_tensor("xT", [B2, DIM, N], BFD, kind="ExternalInput")
    wqkvT_d = nc.dram_tensor("wqkvT", [DIM, 3 * H * DH], BFD, kind="ExternalInput")
    woutT_d = nc.dram_tensor("woutT", [DIM, DIM], BFD, kind="ExternalInput")
    boutB_d = nc.dram_tensor("boutB", [128, DIM], FP32, kind="ExternalInput")
    cosT_d = nc.dram_tensor("cosT", [B2, 128, N], BFD, kind="ExternalInput")
    sinT_d = nc.dram_tensor("sinT", [B2, 128, N], BFD, kind="ExternalInput")
    cstN_d = nc.dram_tensor("cstN", [B2, BLK, NB, 512], BFD, kind="ExternalInput")
    cosE_d = nc.dram_tensor("cosE", [B2, 2, 128, NB], BFD, kind="ExternalInput")
    sinE_d = nc.dram_tensor("sinE", [B2, 2, 128, NB], BFD, kind="ExternalInput")
    transB_d = nc.dram_tensor("transB", [B2, 2, 128, NB], BFD, kind="ExternalInput")

    out_d = nc.dram_tensor("out", [B2, N, DIM], BFD, kind="ExternalOutput")

    swap_mask = []
    for i in range(16):
        swap_mask += [2 * i + 1, 2 * i]

    from contextlib import ExitStack
    with ExitStack() as ctx:
        tc = ctx.enter_context(tile.TileContext(nc))
        ep = ctx.enter_context
        consts = ep(tc.tile_pool(name="consts", bufs=1))
        xT_pool = ep(tc.tile_pool(name="xT", bufs=1))
        wv_pool = ep(tc.tile_pool(name="wv", bufs=1))
        wqk_pool = ep(tc.tile_pool(name="wqk", bufs=1))
        wout_pool = ep(tc.tile_pool(name="wout", bufs=1))
        vb_pool = ep(tc.tile_pool(name="vb", bufs=1))
        qk_pool = ep(tc.tile_pool(name="qk", bufs=2))
        ao_pool = ep(tc.tile_pool(name="ao", bufs=1))
        coef_pool = ep(tc.tile_pool(name="coefs", bufs=1))
        cn_pool = ep(tc.tile_pool(name="cn", bufs=3))
        exp_pool = ep(tc.tile_pool(name="expt", bufs=6))
        tmps_pool = ep(tc.tile_pool(name="tmps", bufs=3))
        tmpb_pool = ep(tc.tile_pool(name="tmpb", bufs=3))
        tmpa_pool = ep(tc.tile_pool(name="tmpa", bufs=2))
        oev_pool = ep(tc.tile_pool(name="oev", bufs=3))
        ps_pool = ep(tc.tile_pool(name="ps", bufs=6, space="PSUM"))
        psp_pool = ep(tc.tile_pool(name="psp", bufs=2, space="PSUM"))
        if True:
            # ---- constants ----
            ones_sb = consts.tile([128, 128], BFD)
            nc.vector.memset(ones_sb, 1.0)
            bout_sb = consts.tile([128, DIM], FP32)
            wout_sb = []

            def load_wout():
                nc.sync.dma_start(out=bout_sb, in_=boutB_d[:, :])
                for jc in range(8):
                    wt = wout_pool.tile([128, DIM], BFD, tag=f"wout{jc}", name=f"wout{jc}")
                    nc.sync.dma_start(out=wt, in_=woutT_d[jc * 128:(jc + 1) * 128, :])
                    wout_sb.append(wt)

            wv_sb = []
            for jsl in range(2):
                wvt = wv_pool.tile([128, 8, 512], BFD, tag=f"wv{jsl}", name=f"wv{jsl}")
                for dk in range(8):
                    nc.sync.dma_start(
                        out=wvt[:, dk, :],
                        in_=wqkvT_d[dk * 128:(dk + 1) * 128,
                                    2048 + jsl * 512: 2048 + (jsl + 1) * 512])
                wv_sb.append(wvt)

            for b in range(B2):
                # ---- load xT for this batch: 8 d-chunk tiles [128, 2048] ----
                xT_sb = []
                for dk in range(8):
                    t = xT_pool.tile([128, N], BFD, tag=f"xT{dk}", name=f"xT{dk}")
                    deng = [nc.sync, nc.gpsimd, nc.scalar][dk % 3]
                    deng.dma_start(
                        out=t, in_=xT_d[b, dk * 128:(dk + 1) * 128, :])
                    xT_sb.append(t)

                # ---- per-batch coefficient tiles ----
                def load_coefs():
                    cosT_sb = coef_pool.tile([128, N], BFD, tag="cosT", name="cosT")
                    sinT_sb = coef_pool.tile([128, N], BFD, tag="sinT", name="sinT")
                    nc.sync.dma_start(out=cosT_sb, in_=cosT_d[b])
                    nc.sync.dma_start(out=sinT_sb, in_=sinT_d[b])
                    cosE_sb, sinE_sb, transB_sb = [], [], []
                    for c2 in range(2):
                        ce = coef_pool.tile([128, NB], BFD, tag=f"cosE{c2}", name=f"cosE{c2}")
                        se = coef_pool.tile([128, NB], BFD, tag=f"sinE{c2}", name=f"sinE{c2}")
                        tb = coef_pool.tile([128, NB], BFD, tag=f"transB{c2}", name=f"transB{c2}")
                        nc.sync.dma_start(out=ce, in_=cosE_d[b, c2])
                        nc.sync.dma_start(out=se, in_=sinE_d[b, c2])
                        nc.sync.dma_start(out=tb, in_=transB_d[b, c2])
                        cosE_sb.append(ce)
                        sinE_sb.append(se)
                        transB_sb.append(tb)
                    return cosT_sb, sinT_sb, cosE_sb, sinE_sb, transB_sb

                # ================= V projection (a-split, natural) ==========
                # Vb store: per J-chunk tile [128, (h,a,dh)=4096]
                vb_sb = []
                for jc in range(4):
                    vt = vb_pool.tile([128, H * BLK * DH], BFD, tag=f"vb{jc}")
                    vb_sb.append(vt)

                for a in range(BLK):
                    for c in range(4):  # J-chunk
                        cst_c = cn_pool.tile([128, 512], BFD, tag="cstN")
                        nc.sync.dma_start(out=cst_c, in_=cstN_d[b, a, c * 128:(c + 1) * 128, :])
                        cn_v = cst_c[:, 0:128].rearrange("p (h i) -> p h i", h=8)
                        sn_v = cst_c[:, 128:256].rearrange("p (h i) -> p h i", h=8)
                        tn_v = cst_c[:, 256:512].rearrange("p (h i t) -> p h i t", h=8, i=16, t=2)

                        for jsl in range(2):  # v column slice (8 heads each)
                            ps = psp_pool.tile([128, 512], FP32, tag="psp")
                            for dk in range(8):
                                lhsT = xT_sb[dk].rearrange(
                                    "p (c j a) -> p c j a", c=4, j=128, a=4)[:, c, :, a]
                                nc.tensor.matmul(
                                    ps, lhsT, wv_sb[jsl][:, dk, :],
                                    start=(dk == 0), stop=(dk == 7))
                            # --- evict (ACT) then rotate + translate on GpSimd ---
                            pvr = tmps_pool.tile([128, 512], BFD, tag="pvr")
                            pv = pvr.rearrange(
                                "p (h half i t) -> p h half i t", h=8, half=2, i=16, t=2)
                            nc.scalar.copy(
                                out=pv[:, :, 1],
                                in_=ps.rearrange(
                                    "p (h half i t) -> p h half i t",
                                    h=8, half=2, i=16, t=2)[:, :, 1])
                            x0 = pv[:, :, 1, :, 0]
                            x1 = pv[:, :, 1, :, 1]
                            dst = vb_sb[c].rearrange(
                                "p (h a half i t) -> p h a half i t",
                                h=16, a=4, half=2, i=16, t=2)
                            hlo, hhi = jsl * 8, (jsl + 1) * 8
                            dflat = dst[:, hlo:hhi, a, 0]
                            de = dst[:, hlo:hhi, a, 1, :, 0]
                            do = dst[:, hlo:hhi, a, 1, :, 1]
                            nc.scalar.copy(out=dflat, in_=ps.rearrange("p (h half i t) -> p h half i t", h=8, half=2, i=16, t=2)[:, :, 0])
                            t0 = tmps_pool.tile([128, 8, 16], BFD, tag="t0")
                            t1 = tmps_pool.tile([128, 8, 16], BFD, tag="t1")
                            t4 = tmps_pool.tile([128, 8, 16], BFD, tag="t4")
                            veng = nc.vector if (a * 4 + c) % 3 else nc.gpsimd
                            veng.tensor_tensor(t0, x0, cn_v, MULT)
                            veng.tensor_tensor(t1, x1, sn_v, MULT)
                            veng.tensor_tensor(t4, t0, t1, SUB)
                            # even_rot = x0 cos - x1 sin + c*trans_even
                            veng.tensor_tensor(de, t4, tn_v[:, :, :, 0], ADD)
                            t2 = tmps_pool.tile([128, 8, 16], BFD, tag="t2")
                            t3 = tmps_pool.tile([128, 8, 16], BFD, tag="t3")
                            t5 = tmps_pool.tile([128, 8, 16], BFD, tag="t5")
                            veng.tensor_tensor(t2, x0, sn_v, MULT)
                            veng.tensor_tensor(t3, x1, cn_v, MULT)
                            veng.tensor_tensor(t5, t2, t3, ADD)
                            veng.tensor_tensor(do, t5, tn_v[:, :, :, 1], ADD)

                # ================= Q/K pairs + attention ====================
                cosT_sb, sinT_sb, cosE_sb, sinE_sb, transB_sb = load_coefs()
                if b == 0:
                    load_wout()
                ao_sb = []
                for c2 in range(8):
                    at = ao_pool.tile([128, N], BFD, tag=f"ao{c2}")
                    ao_sb.append(at)

                def emit_proj(c2):
                    qk_tiles = {}
                    for which, jc in (("q", c2), ("k", 8 + c2)):
                        wq_sb = wqk_pool.tile([128, 8, 128], BFD, tag=f"wqk_{which}")
                        for dk in range(8):
                            nc.sync.dma_start(
                                out=wq_sb[:, dk, :],
                                in_=wqkvT_d[dk * 128:(dk + 1) * 128,
                                            jc * 128:(jc + 1) * 128])
                        qt = qk_pool.tile([128, N], BFD, tag=which)
                        qk_tiles[which] = qt
                        for ts in range(4):
                            ps = ps_pool.tile([128, 512], FP32, tag="ps")
                            for dk in range(8):
                                nc.tensor.matmul(
                                    ps, wq_sb[:, dk, :],
                                    xT_sb[dk][:, ts * 512:(ts + 1) * 512],
                                    start=(dk == 0), stop=(dk == 7))
                            # rot: out = praw*cosT + shuffle(praw)*sinT
                            praw = tmpb_pool.tile([128, 512], BFD, tag="praw")
                            nc.scalar.copy(out=praw, in_=ps)
                            shuf = tmpb_pool.tile([128, 512], BFD, tag="shuf")
                            nc.vector.stream_shuffle(shuf, praw, swap_mask)
                            nc.vector.tensor_tensor(
                                praw, praw, cosT_sb[:, ts * 512:(ts + 1) * 512], MULT)
                            nc.vector.tensor_tensor(
                                shuf, shuf, sinT_sb[:, ts * 512:(ts + 1) * 512], MULT)
                            nc.vector.tensor_tensor(
                                qt[:, ts * 512:(ts + 1) * 512], praw, shuf, ADD)

                    return qk_tiles

                def emit_attn(c2, qk_tiles):
                    # ---- attention per head ----
                    qv = qk_tiles["q"].rearrange("p (i a) -> p i a", a=4)
                    kv = qk_tiles["k"].rearrange("p (c j a) -> p c j a", c=4, j=128, a=4)
                    for hh in range(2):
                        h = 2 * c2 + hh
                        plo, phi = hh * 64, (hh + 1) * 64
                        expts_h = []
                        for Jc in range(4):
                            sim_ps = ps_pool.tile([128, 512], FP32, tag="ps", name="sim")
                            for a in range(BLK):
                                nc.tensor.matmul(
                                    sim_ps,
                                    kv[plo:phi, Jc, :, a],
                                    qv[plo:phi, :, a],
                                    start=(a == 0), stop=(a == 3))
                            et = exp_pool.tile([128, 512], BFD, tag="expt")
                            nc.scalar.activation(
                                out=et, in_=sim_ps,
                                func=mybir.ActivationFunctionType.Exp,
                                scale=SCALE)
                            expts_h.append(et)
                        # PV matmuls do not need the normalizer; start them now
                        pv_pss = []
                        for cp in range(2):
                            pv_ps = ps_pool.tile([128, 512], FP32, tag="ps",
                                                 name=f"pv{cp}")
                            for Jc in range(4):
                                lhsT = vb_sb[Jc].rearrange(
                                    "p (h a d) -> p h a d", h=16, a=4, d=64)[
                                        :, h, 2 * cp:2 * cp + 2, :]
                                nc.tensor.matmul(
                                    pv_ps, lhsT, expts_h[Jc],
                                    start=(Jc == 0), stop=(Jc == 3))
                            pv_pss.append(pv_ps)
                        sums_ps = ps_pool.tile([128, 512], FP32, tag="ps", name="sums")
                        for Jc in range(4):
                            nc.tensor.matmul(
                                sums_ps, ones_sb, expts_h[Jc],
                                start=(Jc == 0), stop=(Jc == 3))
                        nc.scalar.activation(
                            out=sums_ps, in_=sums_ps,
                            func=mybir.ActivationFunctionType.Ln)
                        rsums = tmpa_pool.tile([128, 512], BFD, tag="rsums")
                        nc.scalar.activation(
                            out=rsums, in_=sums_ps,
                            func=mybir.ActivationFunctionType.Exp, scale=-1.0)

                        for cp in range(2):  # d'-chunk (a-pair 2cp, 2cp+1)
                            pv_ps = pv_pss[cp]
                            # normalize, inv-translate, inv-rotate, interleave out
                            asb = tmpa_pool.tile([128, 512], BFD, tag="asb")
                            nc.vector.tensor_tensor(asb, pv_ps, rsums, MULT)
                            a2 = tmpa_pool.tile([128, 512], BFD, tag="a2")
                            nc.vector.tensor_tensor(a2, asb, transB_sb[cp], SUB)
                            shf = tmpa_pool.tile([128, 512], BFD, tag="shf")
                            nc.vector.stream_shuffle(shf, a2, swap_mask)
                            u1 = tmpa_pool.tile([128, 512], BFD, tag="u1")
                            u2 = tmpa_pool.tile([128, 512], BFD, tag="u2")
                            nc.vector.tensor_tensor(u1, a2, cosE_sb[cp], MULT)
                            nc.gpsimd.tensor_tensor(u2, shf, sinE_sb[cp], MULT)
                            aov = ao_sb[c2].rearrange("p (a i) -> p a i", a=4)
                            for ap2 in range(2):
                                nc.vector.tensor_tensor(
                                    aov[plo:phi, 2 * cp + ap2, :],
                                    u1[ap2 * 64:(ap2 + 1) * 64, :],
                                    u2[ap2 * 64:(ap2 + 1) * 64, :],
                                    ADD)

                prev = None
                for c2 in range(8):
                    qk_t = emit_proj(c2)
                    if prev is not None:
                        emit_attn(prev[0], prev[1])
                    prev = (c2, qk_t)
                emit_attn(prev[0], prev[1])

                # ================= output projection ========================
                out_v = out_d[b].rearrange("(i a) e -> i a e", a=4)
                groups = [(a, cI, esl) for a in range(4) for cI in range(4)
                          for esl in range(2)]
                NWAVE = 5
                wave = []
                for gi in range(NWAVE):
                    a, cI, esl = groups[gi]
                    ps = ps_pool.tile([128, 512], FP32, tag="ps", name=f"fw{gi}")
                    for jc in range(7):
                        nc.tensor.matmul(
                            ps,
                            ao_sb[jc][:, a * 512 + cI * 128:
                                      a * 512 + (cI + 1) * 128],
                            wout_sb[jc][:, esl * 512:(esl + 1) * 512],
                            start=(jc == 0), stop=False)
                    wave.append(ps)
                for gi in range(NWAVE):
                    a, cI, esl = groups[gi]
                    nc.tensor.matmul(
                        wave[gi],
                        ao_sb[7][:, a * 512 + cI * 128:a * 512 + (cI + 1) * 128],
                        wout_sb[7][:, esl * 512:(esl + 1) * 512],
                        start=False, stop=True)
                    oev = oev_pool.tile([128, 512], BFD, tag="oev")
                    nc.vector.tensor_tensor(
                        oev, wave[gi], bout_sb[:, esl * 512:(esl + 1) * 512], ADD)
                    nc.sync.dma_start(
                        out=out_v[cI * 128:(cI + 1) * 128, a,
                                  esl * 512:(esl + 1) * 512],
                        in_=oev)
                for gi in range(NWAVE, len(groups)):
                    a, cI, esl = groups[gi]
                    ps = ps_pool.tile([128, 512], FP32, tag="ps")
                    for jc in range(8):
                        nc.tensor.matmul(
                            ps,
                            ao_sb[jc][:, a * 512 + cI * 128:
                                      a * 512 + (cI + 1) * 128],
                            wout_sb[jc][:, esl * 512:(esl + 1) * 512],
                            start=(jc == 0), stop=(jc == 7))
                    oev = oev_pool.tile([128, 512], BFD, tag="oev")
                    nc.vector.tensor_tensor(
                        oev, ps, bout_sb[:, esl * 512:(esl + 1) * 512], ADD)
                    nc.sync.dma_start(
                        out=out_v[cI * 128:(cI + 1) * 128, a,
                                  esl * 512:(esl + 1) * 512],
                        in_=oev)
    _split_multi_waits(nc)
    return nc


def _host_prep(x, angles, trans, W_qkv, W_out, b_out, trans_coeff):
    """Build all per-core input arrays (layout/dtype staging + cos/sin coeffs)."""
    c = float(np.asarray(trans_coeff).reshape(-1)[0])
    cos = np.cos(angles).astype(np.float32)   # [B, N, 16]
    sin = np.sin(angles).astype(np.float32)

    xT = np.ascontiguousarray(x.transpose(0, 2, 1)).astype(BF16)       # [B, DIM, N]
    wqkvT = np.ascontiguousarray(np.asarray(W_qkv).T).astype(BF16)     # [DIM, 3HDH]
    woutT = np.ascontiguousarray(np.asarray(W_out).T).astype(BF16)     # [DIM, DIM]
    boutB = np.ascontiguousarray(
        np.broadcast_to(np.asarray(b_out)[None, :], (128, DIM))).astype(np.float32)

    dh = np.arange(DH)
    pair_idx = np.clip((dh - D_FLAT) // 2, 0, NPAIR - 1)               # [64]
    is_rot = dh >= D_FLAT
    is_odd = ((dh - D_FLAT) % 2 == 1) & is_rot

    # ---- cosT/sinT [B, 128, N]: rows = (half, dh); fwd rotation, [j,t] layout
    base_cos = np.where(is_rot[None, None, :], cos[:, :, pair_idx], 1.0)  # [B,N,64]
    sgn = np.where(is_rot, np.where(is_odd, 1.0, -1.0), 0.0)
    base_sin = sin[:, :, pair_idx] * sgn[None, None, :]
    cosT = np.tile(base_cos.transpose(0, 2, 1), (1, 2, 1)).astype(BF16)   # [B,128,N]
    sinT = np.tile(base_sin.transpose(0, 2, 1), (1, 2, 1)).astype(BF16)

    # ---- cosN/sinN [B, BLK, NB, 128] for V: cols (h=8, i=16)
    J = np.arange(NB)
    cstN = np.empty((B, BLK, NB, 512), np.float32)
    for a in range(BLK):
        t_idx = 4 * J + a
        cstN[:, a, :, 0:128] = np.tile(cos[:, t_idx, :], (1, 1, 8))
        cstN[:, a, :, 128:256] = np.tile(sin[:, t_idx, :], (1, 1, 8))
        cstN[:, a, :, 256:512] = np.tile(c * np.asarray(trans)[:, t_idx, :], (1, 1, 8))
    cstN = cstN.astype(BF16)

    # ---- inverse coeffs [B, 2, 128, NB]: rows = (a2, dh); t = 4I + 2*c2 + a2
    cosE = np.empty((B, 2, 128, NB), np.float32)
    sinE = np.empty((B, 2, 128, NB), np.float32)
    transB = np.zeros((B, 2, 128, NB), np.float32)  # cast to bf16 below
    I = np.arange(NB)
    sgnE = np.where(is_rot, np.where(is_odd, -1.0, 1.0), 0.0)
    for c2 in range(2):
        for a2 in range(2):
            t_idx = 4 * I + 2 * c2 + a2
            cc = cos[:, t_idx, :][:, :, pair_idx].transpose(0, 2, 1)   # [B,64,NB]
            ss = sin[:, t_idx, :][:, :, pair_idx].transpose(0, 2, 1)
            cosE[:, c2, a2 * 64:(a2 + 1) * 64, :] = np.where(
                is_rot[None, :, None], cc, 1.0)
            sinE[:, c2, a2 * 64:(a2 + 1) * 64, :] = ss * sgnE[None, :, None]
            tb = c * np.asarray(trans)[:, t_idx, :].transpose(0, 2, 1)  # [B,32,NB]
            transB[:, c2, a2 * 64 + D_FLAT:(a2 + 1) * 64, :] = tb
    cosE = cosE.astype(BF16)
    sinE = sinE.astype(BF16)

    return dict(xT=xT, wqkvT=wqkvT, woutT=woutT, boutB=boutB,
                cosT=cosT, sinT=sinT, cstN=cstN,
                cosE=cosE, sinE=sinE, transB=transB.astype(BF16))


def kernel(x, angles, trans, W_qkv, W_out, b_out, trans_coeff, _profile=False):
    x = np.asarray(x)
    angles = np.asarray(angles)
    trans = np.asarray(trans)
    arrs = _host_prep(x, angles, trans, W_qkv, W_out, b_out, trans_coeff)
    if "nc" not in _CACHE:
        _CACHE["nc"] = _build_nc()
    nc = _CACHE["nc"]

    in_maps = []
    for core in range(NCORES):
        bsl = slice(core * B2, (core + 1) * B2)
        m = dict(
            xT=np.ascontiguousarray(arrs["xT"][bsl]),
            wqkvT=arrs["wqkvT"], woutT=arrs["woutT"], boutB=arrs["boutB"],
            cosT=np.ascontiguousarray(arrs["cosT"][bsl]),
            sinT=np.ascontiguousarray(arrs["sinT"][bsl]),
            cstN=np.ascontiguousarray(arrs["cstN"][bsl]),
            cosE=np.ascontiguousarray(arrs["cosE"][bsl]),
            sinE=np.ascontiguousarray(arrs["sinE"][bsl]),
            transB=np.ascontiguousarray(arrs["transB"][bsl]),
        )
        in_maps.append(m)

    res = run_bass_kernel_spmd(nc, in_maps, core_ids=list(range(NCORES)),
                               trace=_profile)
    out = np.concatenate([r["out"] for r in res.results], axis=0).astype(np.float32)
    if _profile:
        _CACHE["last_exec_time_ns"] = res.exec_time_ns
        _CACHE["last_trace"] = res.instructions_and_trace
    return out



# revision 2
# speedup vs baseline: 1.0074x; 1.0074x over previous
"""Trainium2 Bass kernel for nn_Attention_65317862638379 — v2.

v2: the three dense projections (QKV, and the output projection) run as fp8
e4m3 DoubleRow matmuls with a 3-term hi/lo error-compensated split
(x_h@W_h + x_l@W_h + x_h@W_l).  In the cost model a DoubleRow fp8 matmul
contracts 2x128 K per instruction at 0.5 cycles/row -> 4x bf16 throughput,
so 3 terms cost 0.75x of bf16.  The attention core (sim/PV/sums) stays bf16.

Sharding: data-parallel over batch, 2 batches per core, 8 cores.
"""

import numpy as np
import ml_dtypes

import concourse.bass as bass
import concourse.mybir as mybir
import concourse.tile as tile
from concourse.bass_utils import run_bass_kernel_spmd

BF16 = ml_dtypes.bfloat16
F8 = ml_dtypes.float8_e4m3fn

B, N, DIM, H, DH = 16, 2048, 1024, 16, 64
D_FLAT, D_ROT, NPAIR = 32, 32, 16
BLK = 4
NB = N // BLK          # 512 block tokens
DB = DH * BLK          # 256 block dim
NCORES = 8
B2 = B // NCORES       # batches per core
SCALE = float((DH * BLK) ** -0.5)  # 1/16, TAU=1.0

FP32 = mybir.dt.float32
BFD = mybir.dt.bfloat16
FP8 = mybir.dt.float8e4
DR = mybir.MatmulPerfMode.DoubleRow

MULT = mybir.AluOpType.mult
ADD = mybir.AluOpType.add
SUB = mybir.AluOpType.subtract

# x and W are host-scaled by 16 each (keeps fp8 hi/lo splits in e4m3 normal
# range); projections therefore come out 256x and are descaled at eviction.
DESC = 1.0 / 256.0
COPY = mybir.ActivationFunctionType.Copy

_CACHE = {}


def _split_multi_waits(nc):
    """walrus codegen only supports one sync-wait per instruction; hoist
    extra waits onto preceding same-engine NoOps."""
    cnt = 0
    for f in nc.m.functions:
        for blk in f.blocks:
            insts = blk.instructions
            out = []
            for inst in insts:
                si = inst.sync_info
                if si is not None and si.on_wait and len(si.on_wait) > 1:
                    waits = list(si.on_wait)
                    for w in waits[:-1]:
                        cnt += 1
                        nop = mybir.InstNoOp(name=f"WSPLIT-{cnt}", ins=[], outs=[])
                        nop.engine = inst.engine
                        nop.sync_info = mybir.SyncInfo(on_wait=[w], on_update=[])
                        out.append(nop)
                    inst.sync_info = mybir.SyncInfo(
                        on_wait=[waits[-1]], on_update=list(si.on_update))
                out.append(inst)
            blk.instructions = out
    return cnt


def _build_nc():
    """Build the Bass graph (SPMD; same NEFF on all 8 cores)."""
    nc = bass.Bass(target_bir_lowering=False)

    # ---------------- DRAM parameters (per-core shapes) ----------------
    # fp8 hi/lo pair layouts: [.., 128 part, 2 (K-chunk pair), cols]
    xPH_d = nc.dram_tensor("xPH", [B2, 4, 128, 2, N], FP8, kind="ExternalInput")
    xPL_d = nc.dram_tensor("xPL", [B2, 4, 128, 2, N], FP8, kind="ExternalInput")
    wqkvPH_d = nc.dram_tensor("wqkvPH", [128, 4, 2, 3 * H * DH], FP8, kind="ExternalInput")
    wqkvPL_d = nc.dram_tensor("wqkvPL", [128, 4, 2, 3 * H * DH], FP8, kind="ExternalInput")
    woutPH_d = nc.dram_tensor("woutPH", [128, 4, 2, DIM], FP8, kind="ExternalInput")
    woutPL_d = nc.dram_tensor("woutPL", [128, 4, 2, DIM], FP8, kind="ExternalInput")
    woutBF_d = nc.dram_tensor("woutBF", [2, 128, DIM], BFD, kind="ExternalInput")
    boutB_d = nc.dram_tensor("boutB", [128, DIM], FP32, kind="ExternalInput")
    cosT_d = nc.dram_tensor("cosT", [B2, 128, N], BFD, kind="ExternalInput")
    sinT_d = nc.dram_tensor("sinT", [B2, 128, N], BFD, kind="ExternalInput")
    cstN_d = nc.dram_tensor("cstN", [B2, BLK, NB, 512], BFD, kind="ExternalInput")
    cosE_d = nc.dram_tensor("cosE", [B2, 2, 128, NB], BFD, kind="ExternalInput")
    sinE_d = nc.dram_tensor("sinE", [B2, 2, 128, NB], BFD, kind="ExternalInput")
    transB_d = nc.dram_tensor("transB", [B2, 2, 128, NB], BFD, kind="ExternalInput")

    out_d = nc.dram_tensor("out", [B2, N, DIM], BFD, kind="ExternalOutput")

    swap_mask = []
    for i in range(16):
        swap_mask += [2 * i + 1, 2 * i]

    from contextlib import ExitStack
    with ExitStack() as ctx:
        tc = ctx.enter_context(tile.TileContext(nc))
        ep = ctx.enter_context
        consts = ep(tc.tile_pool(name="consts", bufs=1))
        x_pool = ep(tc.tile_pool(name="xP", bufs=1))
        wv_pool = ep(tc.tile_pool(name="wv", bufs=1))
        wqk_pool = ep(tc.tile_pool(name="wqk", bufs=1))
        wout_pool = ep(tc.tile_pool(name="wout", bufs=1))
        vb_pool = ep(tc.tile_pool(name="vb", bufs=1))
        qk_pool = ep(tc.tile_pool(name="qk", bufs=2))
        ao_pool = ep(tc.tile_pool(name="ao", bufs=2))
        aoP_pool = ep(tc.tile_pool(name="aoP", bufs=1))
        coef_pool = ep(tc.tile_pool(name="coefs", bufs=1))
        cn_pool = ep(tc.tile_pool(name="cn", bufs=2))
        exp_pool = ep(tc.tile_pool(name="expt", bufs=4))
        tmps_pool = ep(tc.tile_pool(name="tmps", bufs=2))
        praw_pool = ep(tc.tile_pool(name="praw", bufs=2))
        shuf_pool = ep(tc.tile_pool(name="shuf", bufs=1))
        tmpa_pool = ep(tc.tile_pool(name="tmpa", bufs=2))
        oev_pool = ep(tc.tile_pool(name="oev", bufs=2))
        ps_pool = ep(tc.tile_pool(name="ps", bufs=2, space="PSUM"))
        psim_pool = ep(tc.tile_pool(name="psim", bufs=3, space="PSUM"))
        pvs_pool = ep(tc.tile_pool(name="pvs", bufs=3, space="PSUM"))
        if True:
            # ---- constants ----
            ones_sb = consts.tile([128, 128], BFD)
            nc.vector.memset(ones_sb, 1.0)
            bout_sb = consts.tile([128, DIM], FP32)
            wout_sb = []   # [(hi, lo)] x 3 pair-chunks (jc 6,7 run bf16)
            woutBF_sb = []

            def load_wout():
                nc.sync.dma_start(out=bout_sb, in_=boutB_d[:, :])
                for jc in range(2):
                    wb = wout_pool.tile([128, DIM], BFD, tag=f"wobf{jc}", name=f"wobf{jc}")
                    nc.scalar.dma_start(out=wb, in_=woutBF_d[jc])
                    woutBF_sb.append(wb)
                for c in range(3):
                    wh = wout_pool.tile([128, 2, DIM], FP8, tag=f"woh{c}", name=f"woh{c}")
                    wl = wout_pool.tile([128, 2, DIM], FP8, tag=f"wol{c}", name=f"wol{c}")
                    nc.sync.dma_start(out=wh, in_=woutPH_d[:, c])
                    nc.gpsimd.dma_start(out=wl, in_=woutPL_d[:, c])
                    wout_sb.append((wh, wl))

            wv_sb = []     # [(hi, lo)] x 4

            dengs = [nc.sync, nc.gpsimd, nc.scalar]

            def stage_x(b):
                """Allocate batch-b x pair tiles; whole-tile DMAs (one per
                tile) interleaved with the V-weight tiles in first-use order."""
                xh, xl = [], []
                for c in range(4):
                    xh.append(x_pool.tile([128, 2, N], FP8, tag=f"xh{c}", name=f"xh{c}"))
                    xl.append(x_pool.tile([128, 2, N], FP8, tag=f"xl{c}", name=f"xl{c}"))
                dmai = 0
                for c in range(4):
                    dengs[dmai % 3].dma_start(out=xh[c], in_=xPH_d[b, c])
                    dmai += 1
                    if b == 0:
                        dengs[dmai % 3].dma_start(
                            out=wv_sb[c][0], in_=wqkvPH_d[:, c, :, 2048:3072])
                        dmai += 1
                for c in range(4):
                    dengs[dmai % 3].dma_start(out=xl[c], in_=xPL_d[b, c])
                    dmai += 1
                    if b == 0:
                        dengs[dmai % 3].dma_start(
                            out=wv_sb[c][1], in_=wqkvPL_d[:, c, :, 2048:3072])
                        dmai += 1
                return xh, xl

            pend = None
            for b in range(B2):
                if b == 0:
                    for c in range(4):
                        wvh = wv_pool.tile([128, 2, 1024], FP8, tag=f"wvh{c}", name=f"wvh{c}")
                        wvl = wv_pool.tile([128, 2, 1024], FP8, tag=f"wvl{c}", name=f"wvl{c}")
                        wv_sb.append((wvh, wvl))
                    xh_sb, xl_sb = stage_x(0)
                else:
                    xh_sb, xl_sb, vb_sb, coefs = pend
                # ---- per-batch coefficient tiles ----
                def load_coefs(b=b):
                    cosT_sb = coef_pool.tile([128, N], BFD, tag="cosT", name="cosT")
                    sinT_sb = coef_pool.tile([128, N], BFD, tag="sinT", name="sinT")
                    nc.sync.dma_start(out=cosT_sb, in_=cosT_d[b])
                    nc.sync.dma_start(out=sinT_sb, in_=sinT_d[b])
                    cosE_sb, sinE_sb, transB_sb = [], [], []
                    for c2 in range(2):
                        ce = coef_pool.tile([128, NB], BFD, tag=f"cosE{c2}", name=f"cosE{c2}")
                        se = coef_pool.tile([128, NB], BFD, tag=f"sinE{c2}", name=f"sinE{c2}")
                        tb = coef_pool.tile([128, NB], BFD, tag=f"transB{c2}", name=f"transB{c2}")
                        nc.sync.dma_start(out=ce, in_=cosE_d[b, c2])
                        nc.sync.dma_start(out=se, in_=sinE_d[b, c2])
                        nc.sync.dma_start(out=tb, in_=transB_d[b, c2])
                        cosE_sb.append(ce)
                        sinE_sb.append(se)
                        transB_sb.append(tb)
                    return cosT_sb, sinT_sb, cosE_sb, sinE_sb, transB_sb

                # ================= V projection (a-split, natural) ==========
                def build_vproj(vb, vxh, vxl):
                    """Return (vb tiles, list of 16 unit-closures); each unit
                    emits one (a, c) slice of the V projection + rotation."""
                    vb_sb = []
                    for jc in range(4):
                        vb_sb.append(vb_pool.tile([128, H * BLK * DH], BFD, tag=f"vb{jc}", name=f"vb{jc}"))

                    def make_unit(a, c):
                        def unit():
                            cst_c = cn_pool.tile([128, 512], BFD, tag="cstN")
                            nc.sync.dma_start(out=cst_c, in_=cstN_d[vb, a, c * 128:(c + 1) * 128, :])
                            cn_v = cst_c[:, 0:128].rearrange("p (h i) -> p h i", h=8)
                            sn_v = cst_c[:, 128:256].rearrange("p (h i) -> p h i", h=8)
                            tn_v = cst_c[:, 256:512].rearrange("p (h i t) -> p h i t", h=8, i=16, t=2)
                            for jsl in range(2):  # v column slice (8 heads each)
                                vpool, vtag = ((psim_pool, "sim") if (c * 2 + jsl) % 2
                                               else (pvs_pool, "pvs"))
                                ps = vpool.tile([128, 512], FP32, tag=vtag)
                                first = True
                                for ce in range(4):
                                    lhsTh = vxh[ce].rearrange(
                                        "p s (c j a) -> p s c j a", c=4, j=128, a=4)[:, :, c, :, a]
                                    lhsTl = vxl[ce].rearrange(
                                        "p s (c j a) -> p s c j a", c=4, j=128, a=4)[:, :, c, :, a]
                                    wvh = wv_sb[ce][0][:, :, jsl * 512:(jsl + 1) * 512]
                                    wvl = wv_sb[ce][1][:, :, jsl * 512:(jsl + 1) * 512]
                                    for (lh, rh) in ((lhsTh, wvh), (lhsTl, wvh), (lhsTh, wvl)):
                                        nc.tensor.matmul(
                                            ps, lh, rh,
                                            start=first, stop=(ce == 3 and rh is wvl),
                                            perf_mode=DR)
                                        first = False
                                # --- evict (ACT) then rotate + translate ---
                                pvr = tmps_pool.tile([128, 512], BFD, tag="pvr")
                                pv = pvr.rearrange(
                                    "p (h half i t) -> p h half i t", h=8, half=2, i=16, t=2)
                                nc.scalar.activation(
                                    out=pv[:, :, 1],
                                    in_=ps.rearrange(
                                        "p (h half i t) -> p h half i t",
                                        h=8, half=2, i=16, t=2)[:, :, 1],
                                    func=COPY, scale=DESC)
                                x0 = pv[:, :, 1, :, 0]
                                x1 = pv[:, :, 1, :, 1]
                                dst = vb_sb[c].rearrange(
                                    "p (h a half i t) -> p h a half i t",
                                    h=16, a=4, half=2, i=16, t=2)
                                hlo, hhi = jsl * 8, (jsl + 1) * 8
                                dflat = dst[:, hlo:hhi, a, 0]
                                de = dst[:, hlo:hhi, a, 1, :, 0]
                                do = dst[:, hlo:hhi, a, 1, :, 1]
                                nc.scalar.activation(
                                    out=dflat,
                                    in_=ps.rearrange("p (h half i t) -> p h half i t",
                                                     h=8, half=2, i=16, t=2)[:, :, 0],
                                    func=COPY, scale=DESC)
                                t0 = tmps_pool.tile([128, 8, 16], BFD, tag="t0")
                                t1 = tmps_pool.tile([128, 8, 16], BFD, tag="t1")
                                t4 = tmps_pool.tile([128, 8, 16], BFD, tag="t4")
                                veng = nc.vector if (a * 4 + c) % 3 else nc.gpsimd
                                veng.tensor_tensor(t0, x0, cn_v, MULT)
                                veng.tensor_tensor(t1, x1, sn_v, MULT)
                                veng.tensor_tensor(t4, t0, t1, SUB)
                                # even_rot = x0 cos - x1 sin + c*trans_even
                                veng.tensor_tensor(de, t4, tn_v[:, :, :, 0], ADD)
                                t2 = tmps_pool.tile([128, 8, 16], BFD, tag="t2")
                                t3 = tmps_pool.tile([128, 8, 16], BFD, tag="t3")
                                t5 = tmps_pool.tile([128, 8, 16], BFD, tag="t5")
                                veng.tensor_tensor(t2, x0, sn_v, MULT)
                                veng.tensor_tensor(t3, x1, cn_v, MULT)
                                veng.tensor_tensor(t5, t2, t3, ADD)
                                veng.tensor_tensor(do, t5, tn_v[:, :, :, 1], ADD)
                        return unit

                    units = [make_unit(a, c) for c in range(4) for a in range(BLK)]
                    return vb_sb, units

                if b == 0:
                    vb_sb, vunits = build_vproj(0, xh_sb, xl_sb)
                    for u in vunits:
                        u()

                # ================= Q/K pairs + attention ====================
                if b == 0:
                    coefs = load_coefs(0)
                cosT_sb, sinT_sb, cosE_sb, sinE_sb, transB_sb = coefs
                if b == 0:
                    load_wout()
                # ao: bf16 transient per c2 (bufs=2); fp8 hi/lo pair tiles
                aoPh_sb, aoPl_sb = [], []
                for c in range(3):
                    ah = aoP_pool.tile([128, 2, N], FP8, tag=f"aoh{c}", name=f"aoh{c}")
                    al = aoP_pool.tile([128, 2, N], FP8, tag=f"aol{c}", name=f"aol{c}")
                    aoPh_sb.append(ah)
                    aoPl_sb.append(al)

                def emit_proj(c2):
                    qk_tiles = {}
                    for which, jc in (("q", c2), ("k", 8 + c2)):
                        wh_sb = wqk_pool.tile([128, 4, 2, 128], FP8, tag=f"wqk_{which}h")
                        wl_sb = wqk_pool.tile([128, 4, 2, 128], FP8, tag=f"wqk_{which}l")
                        nc.sync.dma_start(
                            out=wh_sb, in_=wqkvPH_d[:, :, :, jc * 128:(jc + 1) * 128])
                        nc.scalar.dma_start(
                            out=wl_sb, in_=wqkvPL_d[:, :, :, jc * 128:(jc + 1) * 128])
                        qt = qk_pool.tile([128, N], BFD, tag=which)
                        qk_tiles[which] = qt
                        praw = praw_pool.tile([128, N], BFD, tag="praw")
                        for ts in range(4):
                            ps = ps_pool.tile([128, 512], FP32, tag="ps")
                            first = True
                            for ce in range(4):
                                xh_r = xh_sb[ce][:, :, ts * 512:(ts + 1) * 512]
                                xl_r = xl_sb[ce][:, :, ts * 512:(ts + 1) * 512]
                                for (wt, xr, last) in ((wh_sb, xh_r, False),
                                                       (wh_sb, xl_r, False),
                                                       (wl_sb, xh_r, ce == 3)):
                                    nc.tensor.matmul(
                                        ps, wt[:, ce], xr,
                                        start=first, stop=last, perf_mode=DR)
                                    first = False
                            nc.scalar.activation(
                                out=praw[:, ts * 512:(ts + 1) * 512], in_=ps,
                                func=COPY, scale=DESC)
                        # rot on the whole tile: qt = praw*cosT + shuf(praw)*sinT
                        shuf = shuf_pool.tile([128, N], BFD, tag="shuf")
                        nc.vector.stream_shuffle(shuf, praw, swap_mask)
                        nc.vector.tensor_tensor(praw, praw, cosT_sb, MULT)
                        nc.vector.tensor_tensor(shuf, shuf, sinT_sb, MULT)
                        nc.vector.tensor_tensor(qt, praw, shuf, ADD)

                    return qk_tiles

                def emit_split(c2):
                    # `at` is 16x-scaled via the host-scaled inverse-rotation
                    # coefficients, so hi/lo need no further scaling
                    at = at_tiles[c2]
                    hslice = aoPh_sb[c2 // 2][:, c2 % 2, :]
                    lslice = aoPl_sb[c2 // 2][:, c2 % 2, :]
                    nc.scalar.activation(out=hslice, in_=at, func=COPY)
                    nc.gpsimd.tensor_tensor(lslice, at, hslice, SUB)

                def emit_attn(c2, qk_tiles):
                    # ---- attention per head ----
                    if 1 <= c2 and c2 - 1 < 6:
                        emit_split(c2 - 1)
                    at = ao_pool.tile([128, N], BFD, tag="ao")
                    qv = qk_tiles["q"].rearrange("p (i a) -> p i a", a=4)
                    kv = qk_tiles["k"].rearrange("p (c j a) -> p c j a", c=4, j=128, a=4)
                    for hh in range(2):
                        h = 2 * c2 + hh
                        plo, phi = hh * 64, (hh + 1) * 64
                        expts_h = []
                        for Jc in range(4):
                            sim_ps = psim_pool.tile([128, 512], FP32, tag="sim", name="sim")
                            for a in range(BLK):
                                nc.tensor.matmul(
                                    sim_ps,
                                    kv[plo:phi, Jc, :, a],
                                    qv[plo:phi, :, a],
                                    start=(a == 0), stop=(a == 3))
                            et = exp_pool.tile([128, 512], BFD, tag="expt")
                            nc.scalar.activation(
                                out=et, in_=sim_ps,
                                func=mybir.ActivationFunctionType.Exp,
                                scale=SCALE)
                            expts_h.append(et)
                        # PV matmuls do not need the normalizer; start them now
                        pv_pss = []
                        for cp in range(2):
                            pv_ps = pvs_pool.tile([128, 512], FP32, tag="pvs",
                                                  name=f"pv{cp}")
                            for Jc in range(4):
                                lhsT = vb_sb[Jc].rearrange(
                                    "p (h a d) -> p h a d", h=16, a=4, d=64)[
                                        :, h, 2 * cp:2 * cp + 2, :]
                                nc.tensor.matmul(
                                    pv_ps, lhsT, expts_h[Jc],
                                    start=(Jc == 0), stop=(Jc == 3))
                            pv_pss.append(pv_ps)
                        sums_ps = pvs_pool.tile([128, 512], FP32, tag="pvs", name="sums")
                        for Jc in range(4):
                            nc.tensor.matmul(
                                sums_ps, ones_sb, expts_h[Jc],
                                start=(Jc == 0), stop=(Jc == 3))
                        nc.scalar.activation(
                            out=sums_ps, in_=sums_ps,
                            func=mybir.ActivationFunctionType.Ln)
                        rsums = tmpa_pool.tile([128, 512], BFD, tag="rsums")
                        nc.scalar.activation(
                            out=rsums, in_=sums_ps,
                            func=mybir.ActivationFunctionType.Exp, scale=-1.0)

                        for cp in range(2):  # d'-chunk (a-pair 2cp, 2cp+1)
                            pv_ps = pv_pss[cp]
                            # normalize, inv-translate, inv-rotate, interleave out
                            asb = tmpa_pool.tile([128, 512], BFD, tag="asb")
                            nc.vector.tensor_tensor(asb, pv_ps, rsums, MULT)
                            a2 = tmpa_pool.tile([128, 512], BFD, tag="a2")
                            nc.vector.tensor_tensor(a2, asb, transB_sb[cp], SUB)
                            shf = tmpa_pool.tile([128, 512], BFD, tag="shf")
                            nc.vector.stream_shuffle(shf, a2, swap_mask)
                            u1 = tmpa_pool.tile([128, 512], BFD, tag="u1")
                            u2 = tmpa_pool.tile([128, 512], BFD, tag="u2")
                            nc.vector.tensor_tensor(u1, a2, cosE_sb[cp], MULT)
                            nc.gpsimd.tensor_tensor(u2, shf, sinE_sb[cp], MULT)
                            aov = at.rearrange("p (a i) -> p a i", a=4)
                            for ap2 in range(2):
                                nc.vector.tensor_tensor(
                                    aov[plo:phi, 2 * cp + ap2, :],
                                    u1[ap2 * 64:(ap2 + 1) * 64, :],
                                    u2[ap2 * 64:(ap2 + 1) * 64, :],
                                    ADD)
                    at_tiles[c2] = at

                # ================= output projection helpers ===============
                out_v = out_d[b].rearrange("(i a) e -> i a e", a=4)
                groups = [(a, cI, esl) for a in range(4) for cI in range(4)
                          for esl in range(2)]

                def emit_group_fp8(ps, gi):
                    a, cI, esl = groups[gi]
                    tsl = slice(a * 512 + cI * 128, a * 512 + (cI + 1) * 128)
                    first = True
                    for c in range(3):
                        aoh = aoPh_sb[c][:, :, tsl]
                        aol = aoPl_sb[c][:, :, tsl]
                        wh = wout_sb[c][0][:, :, esl * 512:(esl + 1) * 512]
                        wl = wout_sb[c][1][:, :, esl * 512:(esl + 1) * 512]
                        for (lh, rh) in ((aoh, wh), (aol, wh), (aoh, wl)):
                            nc.tensor.matmul(ps, lh, rh, start=first, stop=False,
                                             perf_mode=DR)
                            first = False

                def emit_group_tail(ps, gi):
                    # bf16 tail: j-chunks 6,7 read the attn bf16 tiles directly
                    # (weights pre-scaled x256 to match the fp8 terms' scale)
                    a, cI, esl = groups[gi]
                    tsl = slice(a * 512 + cI * 128, a * 512 + (cI + 1) * 128)
                    for jc in range(2):
                        nc.tensor.matmul(
                            ps, at_tiles[6 + jc][:, tsl],
                            woutBF_sb[jc][:, esl * 512:(esl + 1) * 512],
                            start=False, stop=(jc == 1))
                    oevA = oev_pool.tile([128, 512], BFD, tag="oevA")
                    nc.scalar.activation(out=oevA, in_=ps, func=COPY, scale=DESC)
                    oev = oev_pool.tile([128, 512], BFD, tag="oev")
                    nc.vector.tensor_tensor(
                        oev, oevA, bout_sb[:, esl * 512:(esl + 1) * 512], ADD)
                    ndma = 1
                    w2 = 512 // ndma
                    for dh2 in range(ndma):
                        nc.sync.dma_start(
                            out=out_v[cI * 128:(cI + 1) * 128, a,
                                      esl * 512 + dh2 * w2:
                                      esl * 512 + (dh2 + 1) * w2],
                            in_=oev[:, dh2 * w2:(dh2 + 1) * w2])

                # ================= pipeline =================================
                at_tiles = {}
                prev = None
                for c2 in range(8):
                    qk_t = emit_proj(c2)
                    if prev is not None:
                        emit_attn(prev[0], prev[1])
                    prev = (c2, qk_t)
                NW = 2
                wave_ps = []
                for gi in range(NW):
                    ps = ps_pool.tile([128, 512], FP32, tag="ps")
                    emit_group_fp8(ps, gi)
                    wave_ps.append(ps)
                emit_attn(prev[0], prev[1])
                for gi in range(NW):
                    emit_group_tail(wave_ps[gi], gi)
                if b + 1 < B2:
                    # stage batch b+1: x DMAs + coefs now; V-proj units
                    # interleaved with the out-projection groups below
                    xh2, xl2 = stage_x(b + 1)
                    coefs2 = load_coefs(b + 1)
                    vb2, vunits2 = build_vproj(b + 1, xh2, xl2)
                    ui = 0
                    for gi in range(NW, len(groups)):
                        ps = ps_pool.tile([128, 512], FP32, tag="ps")
                        emit_group_fp8(ps, gi)
                        emit_group_tail(ps, gi)
                        if gi % 2 == 1 and ui < len(vunits2):
                            vunits2[ui]()
                            ui += 1
                    while ui < len(vunits2):
                        vunits2[ui]()
                        ui += 1
                    pend = (xh2, xl2, vb2, coefs2)
                else:
                    for gi in range(NW, len(groups)):
                        gpool, gtag = [(ps_pool, "ps"), (psim_pool, "sim"),
                                       (pvs_pool, "pvs")][gi % 3]
                        ps = gpool.tile([128, 512], FP32, tag=gtag)
                        emit_group_fp8(ps, gi)
                        emit_group_tail(ps, gi)
    _split_multi_waits(nc)
    return nc


def _host_prep(x, angles, trans, W_qkv, W_out, b_out, trans_coeff):
    """Build all per-core input arrays (layout/dtype staging + cos/sin coeffs)."""
    c = float(np.asarray(trans_coeff).reshape(-1)[0])
    cos = np.cos(angles).astype(np.float32)   # [B, N, 16]
    sin = np.sin(angles).astype(np.float32)

    def split8(t):
        # 16x scale keeps both hi and lo comfortably in e4m3 normal range
        ts = t * np.float32(16.0)
        h = ts.astype(F8)
        l = (ts - h.astype(np.float32)).astype(F8)
        return h, l

    # x pair layout: [B, 4, 128, 2, N]; [c,p,s,t] = x[t, (2c+s)*128+p]
    xT = np.asarray(x).transpose(0, 2, 1)                  # [B, DIM, N]
    xP = xT.reshape(B, 4, 2, 128, N).transpose(0, 1, 3, 2, 4)  # [B,4,128,2,N]
    xPH, xPL = split8(np.ascontiguousarray(xP))

    # wqkv pair layout: [128, 4, 2, 3072]; [p,c,s,j] = W_qkv[j, (2c+s)*128+p]
    wqkvT = np.asarray(W_qkv).T                            # [DIM, 3HDH]
    wqkvP = wqkvT.reshape(4, 2, 128, 3 * H * DH).transpose(2, 0, 1, 3)
    wqkvPH, wqkvPL = split8(np.ascontiguousarray(wqkvP))

    # wout pair layout: [128, 4, 2, DIM]; [p,c,s,e] = W_out[e, (2c+s)*128+p]
    woutT = np.asarray(W_out).T                            # [HDH, DIM]
    woutP = woutT.reshape(4, 2, 128, DIM).transpose(2, 0, 1, 3)
    woutPH, woutPL = split8(np.ascontiguousarray(woutP))
    # bf16 tail weights for j-chunks 6,7 (x256 to match fp8 term scale)
    woutBF = np.ascontiguousarray(
        (16.0 * woutT[768:1024]).reshape(2, 128, DIM)).astype(BF16)

    boutB = np.ascontiguousarray(
        np.broadcast_to(np.asarray(b_out)[None, :], (128, DIM))).astype(np.float32)

    dh = np.arange(DH)
    pair_idx = np.clip((dh - D_FLAT) // 2, 0, NPAIR - 1)               # [64]
    is_rot = dh >= D_FLAT
    is_odd = ((dh - D_FLAT) % 2 == 1) & is_rot

    # ---- cosT/sinT [B, 128, N]: rows = (half, dh); fwd rotation, [j,t] layout
    base_cos = np.where(is_rot[None, None, :], cos[:, :, pair_idx], 1.0)  # [B,N,64]
    sgn = np.where(is_rot, np.where(is_odd, 1.0, -1.0), 0.0)
    base_sin = sin[:, :, pair_idx] * sgn[None, None, :]
    cosT = np.tile(base_cos.transpose(0, 2, 1), (1, 2, 1)).astype(BF16)   # [B,128,N]
    sinT = np.tile(base_sin.transpose(0, 2, 1), (1, 2, 1)).astype(BF16)

    # ---- cosN/sinN [B, BLK, NB, 128] for V: cols (h=8, i=16)
    J = np.arange(NB)
    cstN = np.empty((B, BLK, NB, 512), np.float32)
    for a in range(BLK):
        t_idx = 4 * J + a
        cstN[:, a, :, 0:128] = np.tile(cos[:, t_idx, :], (1, 1, 8))
        cstN[:, a, :, 128:256] = np.tile(sin[:, t_idx, :], (1, 1, 8))
        cstN[:, a, :, 256:512] = np.tile(c * np.asarray(trans)[:, t_idx, :], (1, 1, 8))
    cstN = cstN.astype(BF16)

    # ---- inverse coeffs [B, 2, 128, NB]: rows = (a2, dh); t = 4I + 2*c2 + a2
    cosE = np.empty((B, 2, 128, NB), np.float32)
    sinE = np.empty((B, 2, 128, NB), np.float32)
    transB = np.zeros((B, 2, 128, NB), np.float32)  # cast to bf16 below
    I = np.arange(NB)
    sgnE = np.where(is_rot, np.where(is_odd, -1.0, 1.0), 0.0)
    for c2 in range(2):
        for a2 in range(2):
            t_idx = 4 * I + 2 * c2 + a2
            cc = cos[:, t_idx, :][:, :, pair_idx].transpose(0, 2, 1)   # [B,64,NB]
            ss = sin[:, t_idx, :][:, :, pair_idx].transpose(0, 2, 1)
            cosE[:, c2, a2 * 64:(a2 + 1) * 64, :] = np.where(
                is_rot[None, :, None], cc, 1.0)
            sinE[:, c2, a2 * 64:(a2 + 1) * 64, :] = ss * sgnE[None, :, None]
            tb = c * np.asarray(trans)[:, t_idx, :].transpose(0, 2, 1)  # [B,32,NB]
            transB[:, c2, a2 * 64 + D_FLAT:(a2 + 1) * 64, :] = tb
    # x16: pre-scales the attention output for its fp8 hi/lo split
    cosE = (16.0 * cosE).astype(BF16)
    sinE = (16.0 * sinE).astype(BF16)

    return dict(xPH=xPH, xPL=xPL, wqkvPH=wqkvPH, wqkvPL=wqkvPL,
                woutPH=woutPH, woutPL=woutPL, woutBF=woutBF, boutB=boutB,
                cosT=cosT, sinT=sinT, cstN=cstN,
                cosE=cosE, sinE=sinE, transB=transB.astype(BF16))


def kernel(x, angles, trans, W_qkv, W_out, b_out, trans_coeff, _profile=False):
    x = np.asarray(x)
    angles = np.asarray(angles)
    trans = np.asarray(trans)
    arrs = _host_prep(x, angles, trans, W_qkv, W_out, b_out, trans_coeff)
    if "nc" not in _CACHE:
        _CACHE["nc"] = _build_nc()
    nc = _CACHE["nc"]

    in_maps = []
    for core in range(NCORES):
        bsl = slice(core * B2, (core + 1) * B2)
        m = dict(
            xPH=np.ascontiguousarray(arrs["xPH"][bsl]),
            xPL=np.ascontiguousarray(arrs["xPL"][bsl]),
            wqkvPH=arrs["wqkvPH"], wqkvPL=arrs["wqkvPL"],
            woutPH=arrs["woutPH"], woutPL=arrs["woutPL"],
            woutBF=arrs["woutBF"], boutB=arrs["boutB"],
            cosT=np.ascontiguousarray(arrs["cosT"][bsl]),
            sinT=np.ascontiguousarray(arrs["sinT"][bsl]),
            cstN=np.ascontiguousarray(arrs["cstN"][bsl]),
            cosE=np.ascontiguousarray(arrs["cosE"][bsl]),
            sinE=np.ascontiguousarray(arrs["sinE"][bsl]),
            transB=np.ascontiguousarray(arrs["transB"][bsl]),
        )
        in_maps.append(m)

    res = run_bass_kernel_spmd(nc, in_maps, core_ids=list(range(NCORES)),
                               trace=_profile)
    out = np.concatenate([r["out"] for r in res.results], axis=0).astype(np.float32)
    if _profile:
        _CACHE["last_exec_time_ns"] = res.exec_time_ns
        _CACHE["last_trace"] = res.instructions_and_trace
    return out


# revision 3
# speedup vs baseline: 1.0150x; 1.0075x over previous
"""Trainium2 Bass kernel for nn_Attention_65317862638379 — v2.

v2: the three dense projections (QKV, and the output projection) run as fp8
e4m3 DoubleRow matmuls with a 3-term hi/lo error-compensated split
(x_h@W_h + x_l@W_h + x_h@W_l).  In the cost model a DoubleRow fp8 matmul
contracts 2x128 K per instruction at 0.5 cycles/row -> 4x bf16 throughput,
so 3 terms cost 0.75x of bf16.  The attention core (sim/PV/sums) stays bf16.

Sharding: data-parallel over batch, 2 batches per core, 8 cores.
"""

import numpy as np
import ml_dtypes

import concourse.bass as bass
import concourse.mybir as mybir
import concourse.tile as tile
from concourse.bass_utils import run_bass_kernel_spmd

BF16 = ml_dtypes.bfloat16
F8 = ml_dtypes.float8_e4m3fn

B, N, DIM, H, DH = 16, 2048, 1024, 16, 64
D_FLAT, D_ROT, NPAIR = 32, 32, 16
BLK = 4
NB = N // BLK          # 512 block tokens
DB = DH * BLK          # 256 block dim
NCORES = 8
B2 = B // NCORES       # batches per core
SCALE = float((DH * BLK) ** -0.5)  # 1/16, TAU=1.0

FP32 = mybir.dt.float32
BFD = mybir.dt.bfloat16
FP8 = mybir.dt.float8e4
DR = mybir.MatmulPerfMode.DoubleRow

MULT = mybir.AluOpType.mult
ADD = mybir.AluOpType.add
SUB = mybir.AluOpType.subtract

# x and W are host-scaled by 16 each (keeps fp8 hi/lo splits in e4m3 normal
# range); projections therefore come out 256x and are descaled at eviction.
DESC = 1.0 / 256.0
COPY = mybir.ActivationFunctionType.Copy

_CACHE = {}


def _split_multi_waits(nc):
    """walrus codegen only supports one sync-wait per instruction; hoist
    extra waits onto preceding same-engine NoOps."""
    cnt = 0
    for f in nc.m.functions:
        for blk in f.blocks:
            insts = blk.instructions
            out = []
            for inst in insts:
                si = inst.sync_info
                if si is not None and si.on_wait and len(si.on_wait) > 1:
                    waits = list(si.on_wait)
                    for w in waits[:-1]:
                        cnt += 1
                        nop = mybir.InstNoOp(name=f"WSPLIT-{cnt}", ins=[], outs=[])
                        nop.engine = inst.engine
                        nop.sync_info = mybir.SyncInfo(on_wait=[w], on_update=[])
                        out.append(nop)
                    inst.sync_info = mybir.SyncInfo(
                        on_wait=[waits[-1]], on_update=list(si.on_update))
                out.append(inst)
            blk.instructions = out
    return cnt


def _build_nc():
    """Build the Bass graph (SPMD; same NEFF on all 8 cores)."""
    nc = bass.Bass(target_bir_lowering=False)

    # ---------------- DRAM parameters (per-core shapes) ----------------
    # fp8 hi/lo pair layouts: [.., 128 part, 2 (K-chunk pair), cols]
    xPH_d = nc.dram_tensor("xPH", [B2, 4, 128, 2, N], FP8, kind="ExternalInput")
    xPL_d = nc.dram_tensor("xPL", [B2, 4, 128, 2, N], FP8, kind="ExternalInput")
    wqkvPH_d = nc.dram_tensor("wqkvPH", [128, 4, 2, 3 * H * DH], FP8, kind="ExternalInput")
    wqkvPL_d = nc.dram_tensor("wqkvPL", [128, 4, 2, 3 * H * DH], FP8, kind="ExternalInput")
    woutPH_d = nc.dram_tensor("woutPH", [128, 4, 2, DIM], FP8, kind="ExternalInput")
    woutPL_d = nc.dram_tensor("woutPL", [128, 4, 2, DIM], FP8, kind="ExternalInput")
    woutBF_d = nc.dram_tensor("woutBF", [2, 128, DIM], BFD, kind="ExternalInput")
    boutB_d = nc.dram_tensor("boutB", [128, DIM], FP32, kind="ExternalInput")
    cosT_d = nc.dram_tensor("cosT", [B2, 128, N], BFD, kind="ExternalInput")
    sinT_d = nc.dram_tensor("sinT", [B2, 128, N], BFD, kind="ExternalInput")
    cstN_d = nc.dram_tensor("cstN", [B2, BLK, NB, 512], BFD, kind="ExternalInput")
    cosE_d = nc.dram_tensor("cosE", [B2, 2, 128, NB], BFD, kind="ExternalInput")
    sinE_d = nc.dram_tensor("sinE", [B2, 2, 128, NB], BFD, kind="ExternalInput")
    transB_d = nc.dram_tensor("transB", [B2, 2, 128, NB], BFD, kind="ExternalInput")

    out_d = nc.dram_tensor("out", [B2, N, DIM], BFD, kind="ExternalOutput")

    swap_mask = []
    for i in range(16):
        swap_mask += [2 * i + 1, 2 * i]

    from contextlib import ExitStack
    with ExitStack() as ctx:
        tc = ctx.enter_context(tile.TileContext(nc))
        ep = ctx.enter_context
        consts = ep(tc.tile_pool(name="consts", bufs=1))
        x_pool = ep(tc.tile_pool(name="xP", bufs=1))
        wv_pool = ep(tc.tile_pool(name="wv", bufs=1))
        wqk_pool = ep(tc.tile_pool(name="wqk", bufs=1))
        wout_pool = ep(tc.tile_pool(name="wout", bufs=1))
        vb_pool = ep(tc.tile_pool(name="vb", bufs=1))
        qk_pool = ep(tc.tile_pool(name="qk", bufs=2))
        ao_pool = ep(tc.tile_pool(name="ao", bufs=2))
        aoP_pool = ep(tc.tile_pool(name="aoP", bufs=1))
        coef_pool = ep(tc.tile_pool(name="coefs", bufs=1))
        cn_pool = ep(tc.tile_pool(name="cn", bufs=2))
        exp_pool = ep(tc.tile_pool(name="expt", bufs=4))
        tmps_pool = ep(tc.tile_pool(name="tmps", bufs=2))
        praw_pool = ep(tc.tile_pool(name="praw", bufs=2))
        shuf_pool = ep(tc.tile_pool(name="shuf", bufs=1))
        tmpa_pool = ep(tc.tile_pool(name="tmpa", bufs=2))
        oev_pool = ep(tc.tile_pool(name="oev", bufs=2))
        ps_pool = ep(tc.tile_pool(name="ps", bufs=2, space="PSUM"))
        psim_pool = ep(tc.tile_pool(name="psim", bufs=3, space="PSUM"))
        pvs_pool = ep(tc.tile_pool(name="pvs", bufs=3, space="PSUM"))
        if True:
            # ---- constants ----
            ones_sb = consts.tile([128, 128], BFD)
            nc.vector.memset(ones_sb, 1.0)
            bout_sb = consts.tile([128, DIM], FP32)
            wout_sb = []   # [(hi, lo)] x 3 pair-chunks (jc 6,7 run bf16)
            woutBF_sb = []

            def load_wout():
                nc.sync.dma_start(out=bout_sb, in_=boutB_d[:, :])
                for jc in range(2):
                    wb = wout_pool.tile([128, DIM], BFD, tag=f"wobf{jc}", name=f"wobf{jc}")
                    nc.scalar.dma_start(out=wb, in_=woutBF_d[jc])
                    woutBF_sb.append(wb)
                for c in range(3):
                    wh = wout_pool.tile([128, 2, DIM], FP8, tag=f"woh{c}", name=f"woh{c}")
                    wl = wout_pool.tile([128, 2, DIM], FP8, tag=f"wol{c}", name=f"wol{c}")
                    nc.sync.dma_start(out=wh, in_=woutPH_d[:, c])
                    nc.gpsimd.dma_start(out=wl, in_=woutPL_d[:, c])
                    wout_sb.append((wh, wl))

            wv_sb = []     # [(hi, lo)] x 4

            dengs = [nc.sync, nc.gpsimd, nc.scalar]

            def stage_x(b):
                """Allocate batch-b x pair tiles; whole-tile DMAs (one per
                tile) interleaved with the V-weight tiles in first-use order."""
                xh, xl = [], []
                for c in range(4):
                    xh.append(x_pool.tile([128, 2, N], FP8, tag=f"xh{c}", name=f"xh{c}"))
                    xl.append(x_pool.tile([128, 2, N], FP8, tag=f"xl{c}", name=f"xl{c}"))
                dmai = 0
                for c in range(4):
                    dengs[dmai % 3].dma_start(out=xh[c], in_=xPH_d[b, c])
                    dmai += 1
                    if b == 0:
                        dengs[dmai % 3].dma_start(
                            out=wv_sb[c][0], in_=wqkvPH_d[:, c, :, 2048:3072])
                        dmai += 1
                for c in range(4):
                    dengs[dmai % 3].dma_start(out=xl[c], in_=xPL_d[b, c])
                    dmai += 1
                    if b == 0:
                        dengs[dmai % 3].dma_start(
                            out=wv_sb[c][1], in_=wqkvPL_d[:, c, :, 2048:3072])
                        dmai += 1
                return xh, xl

            pend = None
            for b in range(B2):
                if b == 0:
                    for c in range(4):
                        wvh = wv_pool.tile([128, 2, 1024], FP8, tag=f"wvh{c}", name=f"wvh{c}")
                        wvl = wv_pool.tile([128, 2, 1024], FP8, tag=f"wvl{c}", name=f"wvl{c}")
                        wv_sb.append((wvh, wvl))
                    xh_sb, xl_sb = stage_x(0)
                else:
                    xh_sb, xl_sb, vb_sb, coefs = pend
                # ---- per-batch coefficient tiles ----
                def load_coefs(b=b):
                    cosT_sb = coef_pool.tile([128, N], BFD, tag="cosT", name="cosT")
                    sinT_sb = coef_pool.tile([128, N], BFD, tag="sinT", name="sinT")
                    nc.sync.dma_start(out=cosT_sb, in_=cosT_d[b])
                    nc.sync.dma_start(out=sinT_sb, in_=sinT_d[b])
                    cosE_sb, sinE_sb, transB_sb = [], [], []
                    for c2 in range(2):
                        ce = coef_pool.tile([128, NB], BFD, tag=f"cosE{c2}", name=f"cosE{c2}")
                        se = coef_pool.tile([128, NB], BFD, tag=f"sinE{c2}", name=f"sinE{c2}")
                        tb = coef_pool.tile([128, NB], BFD, tag=f"transB{c2}", name=f"transB{c2}")
                        nc.sync.dma_start(out=ce, in_=cosE_d[b, c2])
                        nc.sync.dma_start(out=se, in_=sinE_d[b, c2])
                        nc.sync.dma_start(out=tb, in_=transB_d[b, c2])
                        cosE_sb.append(ce)
                        sinE_sb.append(se)
                        transB_sb.append(tb)
                    return cosT_sb, sinT_sb, cosE_sb, sinE_sb, transB_sb

                # ================= V projection (a-split, natural) ==========
                def build_vproj(vb, vxh, vxl):
                    """Return (vb tiles, list of 16 unit-closures); each unit
                    emits one (a, c) slice of the V projection + rotation."""
                    vb_sb = []
                    for jc in range(4):
                        vb_sb.append(vb_pool.tile([128, H * BLK * DH], BFD, tag=f"vb{jc}", name=f"vb{jc}"))

                    def make_unit(a, c):
                        def unit():
                            cst_c = cn_pool.tile([128, 512], BFD, tag="cstN")
                            nc.sync.dma_start(out=cst_c, in_=cstN_d[vb, a, c * 128:(c + 1) * 128, :])
                            cn_v = cst_c[:, 0:128].rearrange("p (h i) -> p h i", h=8)
                            sn_v = cst_c[:, 128:256].rearrange("p (h i) -> p h i", h=8)
                            tn_v = cst_c[:, 256:512].rearrange("p (h i t) -> p h i t", h=8, i=16, t=2)
                            for jsl in range(2):  # v column slice (8 heads each)
                                vpool, vtag = ((psim_pool, "sim") if (c * 2 + jsl) % 2
                                               else (pvs_pool, "pvs"))
                                ps = vpool.tile([128, 512], FP32, tag=vtag)
                                first = True
                                for ce in range(4):
                                    lhsTh = vxh[ce].rearrange(
                                        "p s (c j a) -> p s c j a", c=4, j=128, a=4)[:, :, c, :, a]
                                    lhsTl = vxl[ce].rearrange(
                                        "p s (c j a) -> p s c j a", c=4, j=128, a=4)[:, :, c, :, a]
                                    wvh = wv_sb[ce][0][:, :, jsl * 512:(jsl + 1) * 512]
                                    wvl = wv_sb[ce][1][:, :, jsl * 512:(jsl + 1) * 512]
                                    for (lh, rh) in ((lhsTh, wvh), (lhsTl, wvh), (lhsTh, wvl)):
                                        nc.tensor.matmul(
                                            ps, lh, rh,
                                            start=first, stop=(ce == 3 and rh is wvl),
                                            perf_mode=DR)
                                        first = False
                                # --- evict (ACT) then rotate + translate ---
                                pvr = tmps_pool.tile([128, 512], BFD, tag="pvr")
                                pv = pvr.rearrange(
                                    "p (h half i t) -> p h half i t", h=8, half=2, i=16, t=2)
                                nc.scalar.activation(
                                    out=pv[:, :, 1],
                                    in_=ps.rearrange(
                                        "p (h half i t) -> p h half i t",
                                        h=8, half=2, i=16, t=2)[:, :, 1],
                                    func=COPY, scale=DESC)
                                x0 = pv[:, :, 1, :, 0]
                                x1 = pv[:, :, 1, :, 1]
                                dst = vb_sb[c].rearrange(
                                    "p (h a half i t) -> p h a half i t",
                                    h=16, a=4, half=2, i=16, t=2)
                                hlo, hhi = jsl * 8, (jsl + 1) * 8
                                dflat = dst[:, hlo:hhi, a, 0]
                                de = dst[:, hlo:hhi, a, 1, :, 0]
                                do = dst[:, hlo:hhi, a, 1, :, 1]
                                nc.scalar.activation(
                                    out=dflat,
                                    in_=ps.rearrange("p (h half i t) -> p h half i t",
                                                     h=8, half=2, i=16, t=2)[:, :, 0],
                                    func=COPY, scale=DESC)
                                t0 = tmps_pool.tile([128, 8, 16], BFD, tag="t0")
                                t1 = tmps_pool.tile([128, 8, 16], BFD, tag="t1")
                                t4 = tmps_pool.tile([128, 8, 16], BFD, tag="t4")
                                veng = nc.vector if (a * 4 + c) % 3 else nc.gpsimd
                                veng.tensor_tensor(t0, x0, cn_v, MULT)
                                veng.tensor_tensor(t1, x1, sn_v, MULT)
                                veng.tensor_tensor(t4, t0, t1, SUB)
                                # even_rot = x0 cos - x1 sin + c*trans_even
                                veng.tensor_tensor(de, t4, tn_v[:, :, :, 0], ADD)
                                t2 = tmps_pool.tile([128, 8, 16], BFD, tag="t2")
                                t3 = tmps_pool.tile([128, 8, 16], BFD, tag="t3")
                                t5 = tmps_pool.tile([128, 8, 16], BFD, tag="t5")
                                veng.tensor_tensor(t2, x0, sn_v, MULT)
                                veng.tensor_tensor(t3, x1, cn_v, MULT)
                                veng.tensor_tensor(t5, t2, t3, ADD)
                                veng.tensor_tensor(do, t5, tn_v[:, :, :, 1], ADD)
                        return unit

                    units = [make_unit(a, c) for c in range(4) for a in range(BLK)]
                    return vb_sb, units

                if b == 0:
                    vb_sb, vunits = build_vproj(0, xh_sb, xl_sb)
                    for u in vunits:
                        u()

                # ================= Q/K pairs + attention ====================
                if b == 0:
                    coefs = load_coefs(0)
                cosT_sb, sinT_sb, cosE_sb, sinE_sb, transB_sb = coefs
                if b == 0:
                    load_wout()
                # ao: bf16 transient per c2 (bufs=2); fp8 hi/lo pair tiles
                aoPh_sb, aoPl_sb = [], []
                for c in range(3):
                    ah = aoP_pool.tile([128, 2, N], FP8, tag=f"aoh{c}", name=f"aoh{c}")
                    al = aoP_pool.tile([128, 2, N], FP8, tag=f"aol{c}", name=f"aol{c}")
                    aoPh_sb.append(ah)
                    aoPl_sb.append(al)

                def emit_proj(c2):
                    qk_tiles = {}
                    for which, jc in (("q", c2), ("k", 8 + c2)):
                        wh_sb = wqk_pool.tile([128, 4, 2, 128], FP8, tag=f"wqk_{which}h")
                        wl_sb = wqk_pool.tile([128, 4, 2, 128], FP8, tag=f"wqk_{which}l")
                        nc.sync.dma_start(
                            out=wh_sb, in_=wqkvPH_d[:, :, :, jc * 128:(jc + 1) * 128])
                        nc.scalar.dma_start(
                            out=wl_sb, in_=wqkvPL_d[:, :, :, jc * 128:(jc + 1) * 128])
                        qt = qk_pool.tile([128, N], BFD, tag=which)
                        qk_tiles[which] = qt
                        praw = praw_pool.tile([128, N], BFD, tag="praw")
                        for ts in range(4):
                            ps = ps_pool.tile([128, 512], FP32, tag="ps")
                            first = True
                            for ce in range(4):
                                xh_r = xh_sb[ce][:, :, ts * 512:(ts + 1) * 512]
                                xl_r = xl_sb[ce][:, :, ts * 512:(ts + 1) * 512]
                                for (wt, xr, last) in ((wh_sb, xh_r, False),
                                                       (wh_sb, xl_r, False),
                                                       (wl_sb, xh_r, ce == 3)):
                                    nc.tensor.matmul(
                                        ps, wt[:, ce], xr,
                                        start=first, stop=last, perf_mode=DR)
                                    first = False
                            nc.scalar.activation(
                                out=praw[:, ts * 512:(ts + 1) * 512], in_=ps,
                                func=COPY, scale=DESC)
                        # rot on the whole tile: qt = praw*cosT + shuf(praw)*sinT
                        shuf = shuf_pool.tile([128, N], BFD, tag="shuf")
                        nc.vector.stream_shuffle(shuf, praw, swap_mask)
                        nc.vector.tensor_tensor(praw, praw, cosT_sb, MULT)
                        nc.vector.tensor_tensor(shuf, shuf, sinT_sb, MULT)
                        nc.vector.tensor_tensor(qt, praw, shuf, ADD)

                    return qk_tiles

                def emit_split(c2):
                    # `at` is 16x-scaled via the host-scaled inverse-rotation
                    # coefficients, so hi/lo need no further scaling
                    at = at_tiles[c2]
                    hslice = aoPh_sb[c2 // 2][:, c2 % 2, :]
                    lslice = aoPl_sb[c2 // 2][:, c2 % 2, :]
                    nc.scalar.activation(out=hslice, in_=at, func=COPY)
                    nc.gpsimd.tensor_tensor(lslice, at, hslice, SUB)

                def emit_attn(c2, qk_tiles):
                    # ---- attention per head ----
                    if 1 <= c2 and c2 - 1 < 6:
                        emit_split(c2 - 1)
                    at = ao_pool.tile([128, N], BFD, tag="ao")
                    qv = qk_tiles["q"].rearrange("p (i a) -> p i a", a=4)
                    kv = qk_tiles["k"].rearrange("p (c j a) -> p c j a", c=4, j=128, a=4)
                    for hh in range(2):
                        h = 2 * c2 + hh
                        plo, phi = hh * 64, (hh + 1) * 64
                        expts_h = []
                        for Jc in range(4):
                            sim_ps = psim_pool.tile([128, 512], FP32, tag="sim", name="sim")
                            for a in range(BLK):
                                nc.tensor.matmul(
                                    sim_ps,
                                    kv[plo:phi, Jc, :, a],
                                    qv[plo:phi, :, a],
                                    start=(a == 0), stop=(a == 3))
                            et = exp_pool.tile([128, 512], BFD, tag="expt")
                            nc.scalar.activation(
                                out=et, in_=sim_ps,
                                func=mybir.ActivationFunctionType.Exp,
                                scale=SCALE)
                            expts_h.append(et)
                        # PV matmuls do not need the normalizer; start them now
                        pv_pss = []
                        for cp in range(2):
                            pv_ps = pvs_pool.tile([128, 512], FP32, tag="pvs",
                                                  name=f"pv{cp}")
                            for Jc in range(4):
                                lhsT = vb_sb[Jc].rearrange(
                                    "p (h a d) -> p h a d", h=16, a=4, d=64)[
                                        :, h, 2 * cp:2 * cp + 2, :]
                                nc.tensor.matmul(
                                    pv_ps, lhsT, expts_h[Jc],
                                    start=(Jc == 0), stop=(Jc == 3))
                            pv_pss.append(pv_ps)
                        sums_ps = pvs_pool.tile([128, 512], FP32, tag="pvs", name="sums")
                        for Jc in range(4):
                            nc.tensor.matmul(
                                sums_ps, ones_sb, expts_h[Jc],
                                start=(Jc == 0), stop=(Jc == 3))
                        nc.scalar.activation(
                            out=sums_ps, in_=sums_ps,
                            func=mybir.ActivationFunctionType.Ln)
                        rsums = tmpa_pool.tile([128, 512], BFD, tag="rsums")
                        nc.scalar.activation(
                            out=rsums, in_=sums_ps,
                            func=mybir.ActivationFunctionType.Exp, scale=-1.0)

                        for cp in range(2):  # d'-chunk (a-pair 2cp, 2cp+1)
                            pv_ps = pv_pss[cp]
                            # normalize, inv-translate, inv-rotate, interleave out
                            asb = tmpa_pool.tile([128, 512], BFD, tag="asb")
                            nc.vector.tensor_tensor(asb, pv_ps, rsums, MULT)
                            a2 = tmpa_pool.tile([128, 512], BFD, tag="a2")
                            nc.vector.tensor_tensor(a2, asb, transB_sb[cp], SUB)
                            shf = tmpa_pool.tile([128, 512], BFD, tag="shf")
                            nc.vector.stream_shuffle(shf, a2, swap_mask)
                            u1 = tmpa_pool.tile([128, 512], BFD, tag="u1")
                            u2 = tmpa_pool.tile([128, 512], BFD, tag="u2")
                            nc.vector.tensor_tensor(u1, a2, cosE_sb[cp], MULT)
                            nc.gpsimd.tensor_tensor(u2, shf, sinE_sb[cp], MULT)
                            aov = at.rearrange("p (a i) -> p a i", a=4)
                            for ap2 in range(2):
                                nc.vector.tensor_tensor(
                                    aov[plo:phi, 2 * cp + ap2, :],
                                    u1[ap2 * 64:(ap2 + 1) * 64, :],
                                    u2[ap2 * 64:(ap2 + 1) * 64, :],
                                    ADD)
                    at_tiles[c2] = at

                # ================= output projection helpers ===============
                out_v = out_d[b].rearrange("(i a) e -> i a e", a=4)
                groups = [(a, cI, esl) for a in range(4) for cI in range(4)
                          for esl in range(2)]

                def emit_group_fp8(ps, gi):
                    a, cI, esl = groups[gi]
                    tsl = slice(a * 512 + cI * 128, a * 512 + (cI + 1) * 128)
                    first = True
                    for c in range(3):
                        aoh = aoPh_sb[c][:, :, tsl]
                        aol = aoPl_sb[c][:, :, tsl]
                        wh = wout_sb[c][0][:, :, esl * 512:(esl + 1) * 512]
                        wl = wout_sb[c][1][:, :, esl * 512:(esl + 1) * 512]
                        for (lh, rh) in ((aoh, wh), (aol, wh), (aoh, wl)):
                            nc.tensor.matmul(ps, lh, rh, start=first, stop=False,
                                             perf_mode=DR)
                            first = False

                def emit_group_tail(ps, gi):
                    # bf16 tail: j-chunks 6,7 read the attn bf16 tiles directly
                    # (weights pre-scaled x256 to match the fp8 terms' scale)
                    a, cI, esl = groups[gi]
                    tsl = slice(a * 512 + cI * 128, a * 512 + (cI + 1) * 128)
                    for jc in range(2):
                        nc.tensor.matmul(
                            ps, at_tiles[6 + jc][:, tsl],
                            woutBF_sb[jc][:, esl * 512:(esl + 1) * 512],
                            start=False, stop=(jc == 1))
                    oevA = oev_pool.tile([128, 512], BFD, tag="oevA")
                    nc.scalar.activation(out=oevA, in_=ps, func=COPY, scale=DESC)
                    oev = oev_pool.tile([128, 512], BFD, tag="oev")
                    nc.vector.tensor_tensor(
                        oev, oevA, bout_sb[:, esl * 512:(esl + 1) * 512], ADD)
                    ndma = 1
                    w2 = 512 // ndma
                    for dh2 in range(ndma):
                        nc.sync.dma_start(
                            out=out_v[cI * 128:(cI + 1) * 128, a,
                                      esl * 512 + dh2 * w2:
                                      esl * 512 + (dh2 + 1) * w2],
                            in_=oev[:, dh2 * w2:(dh2 + 1) * w2])

                # ================= pipeline =================================
                at_tiles = {}
                prev = None
                for c2 in range(8):
                    qk_t = emit_proj(c2)
                    if prev is not None:
                        emit_attn(prev[0], prev[1])
                    prev = (c2, qk_t)
                NW = 0
                wave_ps = []
                for gi in range(NW):
                    ps = ps_pool.tile([128, 512], FP32, tag="ps")
                    emit_group_fp8(ps, gi)
                    wave_ps.append(ps)
                emit_attn(prev[0], prev[1])
                for gi in range(NW):
                    emit_group_tail(wave_ps[gi], gi)
                if b + 1 < B2:
                    # stage batch b+1: x DMAs + coefs now; V-proj units
                    # interleaved with the out-projection groups below
                    xh2, xl2 = stage_x(b + 1)
                    coefs2 = load_coefs(b + 1)
                    vb2, vunits2 = build_vproj(b + 1, xh2, xl2)
                    ui = 0
                    for gi in range(NW, len(groups)):
                        ps = ps_pool.tile([128, 512], FP32, tag="ps")
                        emit_group_fp8(ps, gi)
                        emit_group_tail(ps, gi)
                        if gi % 2 == 1 and ui < len(vunits2):
                            vunits2[ui]()
                            ui += 1
                    while ui < len(vunits2):
                        vunits2[ui]()
                        ui += 1
                    pend = (xh2, xl2, vb2, coefs2)
                else:
                    for gi in range(NW, len(groups)):
                        gpool, gtag = [(ps_pool, "ps"), (psim_pool, "sim"),
                                       (pvs_pool, "pvs")][gi % 3]
                        ps = gpool.tile([128, 512], FP32, tag=gtag)
                        emit_group_fp8(ps, gi)
                        emit_group_tail(ps, gi)
    _split_multi_waits(nc)
    return nc


def _host_prep(x, angles, trans, W_qkv, W_out, b_out, trans_coeff):
    """Build all per-core input arrays (layout/dtype staging + cos/sin coeffs)."""
    c = float(np.asarray(trans_coeff).reshape(-1)[0])
    cos = np.cos(angles).astype(np.float32)   # [B, N, 16]
    sin = np.sin(angles).astype(np.float32)

    def split8(t):
        # 16x scale keeps both hi and lo comfortably in e4m3 normal range
        ts = t * np.float32(16.0)
        h = ts.astype(F8)
        l = (ts - h.astype(np.float32)).astype(F8)
        return h, l

    # x pair layout: [B, 4, 128, 2, N]; [c,p,s,t] = x[t, (2c+s)*128+p]
    xT = np.asarray(x).transpose(0, 2, 1)                  # [B, DIM, N]
    xP = xT.reshape(B, 4, 2, 128, N).transpose(0, 1, 3, 2, 4)  # [B,4,128,2,N]
    xPH, xPL = split8(np.ascontiguousarray(xP))

    # wqkv pair layout: [128, 4, 2, 3072]; [p,c,s,j] = W_qkv[j, (2c+s)*128+p]
    wqkvT = np.asarray(W_qkv).T                            # [DIM, 3HDH]
    wqkvP = wqkvT.reshape(4, 2, 128, 3 * H * DH).transpose(2, 0, 1, 3)
    wqkvPH, wqkvPL = split8(np.ascontiguousarray(wqkvP))

    # wout pair layout: [128, 4, 2, DIM]; [p,c,s,e] = W_out[e, (2c+s)*128+p]
    woutT = np.asarray(W_out).T                            # [HDH, DIM]
    woutP = woutT.reshape(4, 2, 128, DIM).transpose(2, 0, 1, 3)
    woutPH, woutPL = split8(np.ascontiguousarray(woutP))
    # bf16 tail weights for j-chunks 6,7 (x256 to match fp8 term scale)
    woutBF = np.ascontiguousarray(
        (16.0 * woutT[768:1024]).reshape(2, 128, DIM)).astype(BF16)

    boutB = np.ascontiguousarray(
        np.broadcast_to(np.asarray(b_out)[None, :], (128, DIM))).astype(np.float32)

    dh = np.arange(DH)
    pair_idx = np.clip((dh - D_FLAT) // 2, 0, NPAIR - 1)               # [64]
    is_rot = dh >= D_FLAT
    is_odd = ((dh - D_FLAT) % 2 == 1) & is_rot

    # ---- cosT/sinT [B, 128, N]: rows = (half, dh); fwd rotation, [j,t] layout
    base_cos = np.where(is_rot[None, None, :], cos[:, :, pair_idx], 1.0)  # [B,N,64]
    sgn = np.where(is_rot, np.where(is_odd, 1.0, -1.0), 0.0)
    base_sin = sin[:, :, pair_idx] * sgn[None, None, :]
    cosT = np.tile(base_cos.transpose(0, 2, 1), (1, 2, 1)).astype(BF16)   # [B,128,N]
    sinT = np.tile(base_sin.transpose(0, 2, 1), (1, 2, 1)).astype(BF16)

    # ---- cosN/sinN [B, BLK, NB, 128] for V: cols (h=8, i=16)
    J = np.arange(NB)
    cstN = np.empty((B, BLK, NB, 512), np.float32)
    for a in range(BLK):
        t_idx = 4 * J + a
        cstN[:, a, :, 0:128] = np.tile(cos[:, t_idx, :], (1, 1, 8))
        cstN[:, a, :, 128:256] = np.tile(sin[:, t_idx, :], (1, 1, 8))
        cstN[:, a, :, 256:512] = np.tile(c * np.asarray(trans)[:, t_idx, :], (1, 1, 8))
    cstN = cstN.astype(BF16)

    # ---- inverse coeffs [B, 2, 128, NB]: rows = (a2, dh); t = 4I + 2*c2 + a2
    cosE = np.empty((B, 2, 128, NB), np.float32)
    sinE = np.empty((B, 2, 128, NB), np.float32)
    transB = np.zeros((B, 2, 128, NB), np.float32)  # cast to bf16 below
    I = np.arange(NB)
    sgnE = np.where(is_rot, np.where(is_odd, -1.0, 1.0), 0.0)
    for c2 in range(2):
        for a2 in range(2):
            t_idx = 4 * I + 2 * c2 + a2
            cc = cos[:, t_idx, :][:, :, pair_idx].transpose(0, 2, 1)   # [B,64,NB]
            ss = sin[:, t_idx, :][:, :, pair_idx].transpose(0, 2, 1)
            cosE[:, c2, a2 * 64:(a2 + 1) * 64, :] = np.where(
                is_rot[None, :, None], cc, 1.0)
            sinE[:, c2, a2 * 64:(a2 + 1) * 64, :] = ss * sgnE[None, :, None]
            tb = c * np.asarray(trans)[:, t_idx, :].transpose(0, 2, 1)  # [B,32,NB]
            transB[:, c2, a2 * 64 + D_FLAT:(a2 + 1) * 64, :] = tb
    # x16: pre-scales the attention output for its fp8 hi/lo split
    cosE = (16.0 * cosE).astype(BF16)
    sinE = (16.0 * sinE).astype(BF16)

    return dict(xPH=xPH, xPL=xPL, wqkvPH=wqkvPH, wqkvPL=wqkvPL,
                woutPH=woutPH, woutPL=woutPL, woutBF=woutBF, boutB=boutB,
                cosT=cosT, sinT=sinT, cstN=cstN,
                cosE=cosE, sinE=sinE, transB=transB.astype(BF16))


def kernel(x, angles, trans, W_qkv, W_out, b_out, trans_coeff, _profile=False):
    x = np.asarray(x)
    angles = np.asarray(angles)
    trans = np.asarray(trans)
    arrs = _host_prep(x, angles, trans, W_qkv, W_out, b_out, trans_coeff)
    if "nc" not in _CACHE:
        _CACHE["nc"] = _build_nc()
    nc = _CACHE["nc"]

    in_maps = []
    for core in range(NCORES):
        bsl = slice(core * B2, (core + 1) * B2)
        m = dict(
            xPH=np.ascontiguousarray(arrs["xPH"][bsl]),
            xPL=np.ascontiguousarray(arrs["xPL"][bsl]),
            wqkvPH=arrs["wqkvPH"], wqkvPL=arrs["wqkvPL"],
            woutPH=arrs["woutPH"], woutPL=arrs["woutPL"],
            woutBF=arrs["woutBF"], boutB=arrs["boutB"],
            cosT=np.ascontiguousarray(arrs["cosT"][bsl]),
            sinT=np.ascontiguousarray(arrs["sinT"][bsl]),
            cstN=np.ascontiguousarray(arrs["cstN"][bsl]),
            cosE=np.ascontiguousarray(arrs["cosE"][bsl]),
            sinE=np.ascontiguousarray(arrs["sinE"][bsl]),
            transB=np.ascontiguousarray(arrs["transB"][bsl]),
        )
        in_maps.append(m)

    res = run_bass_kernel_spmd(nc, in_maps, core_ids=list(range(NCORES)),
                               trace=_profile)
    out = np.concatenate([r["out"] for r in res.results], axis=0).astype(np.float32)
    if _profile:
        _CACHE["last_exec_time_ns"] = res.exec_time_ns
        _CACHE["last_trace"] = res.instructions_and_trace
    return out
